# revision 15
# baseline (speedup 1.0000x reference)
"""Trainium2 Bass kernel for nn_LookupTableLayer (embedding_lookup).

Full-input contract: kernel(**inputs) takes the full unsharded numpy inputs,
shards positions across 8 NeuronCores (batch dim), runs one SPMD NEFF on
cores 0-7, and returns the full [16, 512, 32, 128] f32 output.

Algorithm:
  out[..., 0:64] = pairsum(tx')[idx0], out[..., 64:128] = pairsum(ty')[idx1]
  where t' = 0.1*(t/max(t)) + fixed_table. Pair-summed 64-wide f32 tables are
  precomputed on-chip (tile-managed preproc), stored to DRAM scratch, then
  gathered with per-column [128,1]-offset SWDGE indirect DMAs (the only offset
  shape the HW ucode walks correctly).

  The gather loop is RAW bass (no TileContext): tile inserts a semaphore wait
  on every gather (~310 ns/instr extra); raw back-to-back issue relies on
  ucode-side carveout reclaim (cf. concourse/benchmark/swdge_reclaim_perf.py)
  with one coarse wait per chunk for the consumers.
"""

import numpy as np

import concourse.bacc as bacc
import concourse.bass as bass
import concourse.bass_isa as bass_isa
import concourse.mybir as mybir
import concourse.tile as tile
from concourse.bass_utils import run_bass_kernel_spmd

N_CORES = 8
B, M, R, D = 16, 512, 32, 128
TABLE_LEN = 4106
T = (B // N_CORES) * M * R  # 32768 tokens per core
PAIRS = D // 2  # 64
FLAT_N = TABLE_LEN * D // 128  # 4106
PAIR_N = FLAT_N // 2  # 2053
CHUNK = 8192
NCHUNK = T // CHUNK  # 4
C = CHUNK // 128  # 64 tokens per partition per chunk

F32 = mybir.dt.float32
I32 = mybir.dt.int32


def _flat(h, p):
    return h[:].rearrange("a b -> (a b)").rearrange("(p n) -> p n", p=p)


def build_nc():
    nc = bacc.Bacc("TRN2", target_bir_lowering=False, debug=False)
    pos = nc.dram_tensor("positions", [T, 2], I32, kind="ExternalInput")
    fixed = nc.dram_tensor("fixed_table", [TABLE_LEN, D], F32, kind="ExternalInput")
    tx = nc.dram_tensor("table_x", [TABLE_LEN, D], F32, kind="ExternalInput")
    ty = nc.dram_tensor("table_y", [TABLE_LEN, D], F32, kind="ExternalInput")
    out = nc.dram_tensor("out", [T, D], F32, kind="ExternalOutput")
    txp_d = nc.dram_tensor("txp", [TABLE_LEN, PAIRS], F32, kind="Internal")
    typ_d = nc.dram_tensor("typ", [TABLE_LEN, PAIRS], F32, kind="Internal")

    # ---- tile-managed preproc (exits with a full drain barrier, so the
    # scratch tables are complete in DRAM before the raw loop below runs)
    with tile.TileContext(nc) as tc:
        with tc.tile_pool(name="prep", bufs=1) as prep:
            xt = prep.tile([128, FLAT_N], F32)
            yt = prep.tile([128, FLAT_N], F32)
            ft = prep.tile([128, FLAT_N], F32)
            nc.sync.dma_start(xt[:], _flat(tx, 128))
            nc.sync.dma_start(yt[:], _flat(ty, 128))
            nc.sync.dma_start(ft[:], _flat(fixed, 128))

            fp = prep.tile([128, PAIR_N], F32)
            fr = ft[:].rearrange("p (n two) -> p n two", two=2)
            nc.vector.tensor_add(fp[:], fr[:, :, 0], fr[:, :, 1])

            for src_t, dram in ((xt, txp_d), (yt, typ_d)):
                mx = prep.tile([128, 1], F32, tag="mx")
                nc.vector.reduce_max(mx[:], src_t[:], axis=mybir.AxisListType.X)
                gm = prep.tile([128, 1], F32, tag="gm")
                nc.gpsimd.partition_all_reduce(gm[:], mx[:], 128, bass_isa.ReduceOp.max)
                sc = prep.tile([128, 1], F32, tag="sc")
                nc.vector.reciprocal(sc[:], gm[:])
                nc.vector.tensor_scalar_mul(sc[:], sc[:], 0.1)
                pr = src_t[:].rearrange("p (n two) -> p n two", two=2)
                ps = prep.tile([128, PAIR_N], F32, tag="ps")
                nc.vector.tensor_add(ps[:], pr[:, :, 0], pr[:, :, 1])
                nc.vector.scalar_tensor_tensor(
                    ps[:], ps[:], sc[:, 0:1], fp[:],
                    op0=mybir.AluOpType.mult, op1=mybir.AluOpType.add,
                )
                nc.sync.dma_start(_flat(dram, 128), ps[:])

            # belt-and-suspenders DRAM RAW fence (readback on the same HWDGE
            # ring + gpsimd pin); the tc-exit drain also covers this.
            chk = prep.tile([128, 4], F32, tag="chk")
            nc.sync.dma_start(chk[:], _flat(typ_d, 128)[:, 0:4])
            chk2 = prep.tile([128, 4], F32, tag="chk2")
            nc.gpsimd.tensor_copy(chk2[:], chk[:])

    # ---- raw main loop: double-buffered chunks, coarse per-chunk sems
    posc = [
        nc.alloc_sbuf_tensor(f"posc{i}", [128, C, 2], I32)
        for i in range(2)
    ]
    gx = [
        nc.alloc_sbuf_tensor(f"gx{i}", [128, C, PAIRS], F32)
        for i in range(2)
    ]
    gy = [
        nc.alloc_sbuf_tensor(f"gy{i}", [128, C, PAIRS], F32)
        for i in range(2)
    ]
    mg = [
        nc.alloc_sbuf_tensor(f"mg{i}", [128, C, D], F32)
        for i in range(2)
    ]
    s_pos = nc.alloc_semaphore("s_pos")
    s_g = [nc.alloc_semaphore(f"s_g{i}") for i in range(2)]
    s_m = nc.alloc_semaphore("s_m")
    s_st = nc.alloc_semaphore("s_st")

    def pos_load(k):
        nc.sync.dma_start(
            posc[k % 2][:],
            pos[k * CHUNK : (k + 1) * CHUNK, :].rearrange(
                "(p c) two -> p c two", p=128
            ),
        ).then_inc(s_pos, 16)

    # sync stream: loads 0,1 up front; k>=2 loads gated on chunk k-2 gathers
    # (gather completion implies the offsets in posc[k%2] were consumed);
    # stores gated on merges.
    pos_load(0)
    pos_load(1)
    nc.sync.wait_ge(s_g[0], 2048)
    pos_load(2)
    nc.sync.wait_ge(s_g[1], 2048)
    pos_load(3)
    for k in range(NCHUNK):
        nc.sync.wait_ge(s_m, k + 1)
        nc.sync.dma_start(
            out[k * CHUNK : (k + 1) * CHUNK, :].rearrange(
                "(p c) f -> p (c f)", p=128
            ),
            mg[k % 2][:].rearrange("p c f -> p (c f)"),
        ).then_inc(s_st, 16)
    nc.sync.wait_ge(s_st, 16 * NCHUNK)

    # gpsimd stream: back-to-back gathers, no per-instruction waits
    for k in range(NCHUNK):
        nc.gpsimd.wait_ge(s_pos, 16 * (k + 1))
        if k >= 2:
            nc.gpsimd.wait_ge(s_m, k - 1)  # merge k-2 done: gx/gy reusable
        for c in range(C):
            nc.gpsimd.indirect_dma_start(
                out=gx[k % 2][:, c, :],
                out_offset=None,
                in_=txp_d[:],
                in_offset=bass.IndirectOffsetOnAxis(
                    ap=posc[k % 2][:, c, 0:1], axis=0
                ),
            ).then_inc(s_g[k % 2], 16)
            nc.gpsimd.indirect_dma_start(
                out=gy[k % 2][:, c, :],
                out_offset=None,
                in_=typ_d[:],
                in_offset=bass.IndirectOffsetOnAxis(
                    ap=posc[k % 2][:, c, 1:2], axis=0
                ),
            ).then_inc(s_g[k % 2], 16)

    # vector stream: merge per chunk after its gathers complete
    for k in range(NCHUNK):
        nc.vector.wait_ge(s_g[k % 2], 2048 * (k // 2 + 1))
        if k >= 2:
            nc.vector.wait_ge(s_st, 16 * (k - 1))  # store k-2 done: mg reusable
        nc.vector.tensor_copy(mg[k % 2][:, :, 0:PAIRS], gx[k % 2][:])
        nc.vector.tensor_copy(mg[k % 2][:, :, PAIRS:D], gy[k % 2][:]).then_inc(
            s_m, 1
        )

    nc.compile()
    return nc


_cache = {}


def kernel(positions, fixed_table, table_x, table_y):
    nc = _cache.get("nc")
    if nc is None:
        nc = _cache["nc"] = build_nc()
    pos_flat = np.ascontiguousarray(positions.reshape(-1, 2))
    shards = np.split(pos_flat, N_CORES, axis=0)
    fixed_table = np.ascontiguousarray(fixed_table, dtype=np.float32)
    table_x = np.ascontiguousarray(table_x, dtype=np.float32)
    table_y = np.ascontiguousarray(table_y, dtype=np.float32)
    in_maps = [
        {
            "positions": np.ascontiguousarray(s),
            "fixed_table": fixed_table,
            "table_x": table_x,
            "table_y": table_y,
        }
        for s in shards
    ]
    res = run_bass_kernel_spmd(nc, in_maps, core_ids=list(range(N_CORES)))
    outs = [r["out"] for r in res.results]
    return np.concatenate(outs, axis=0).reshape(B, M, R, D)
